# revision 2
# baseline (speedup 1.0000x reference)
"""Trainium2 Bass kernel for nn_Degrade: depthwise 13x13 blur + 4x downsample.

im [8,4,1024,1024] f32, kernel [8,1,13,13] f32 -> out [8,4,256,256] f32.
Sharding: pure data parallel, one sample per NeuronCore (8 cores).

Design: 4x column-tiled PE + fp8 image + fp16 weights (mixed-dtype matmul).
  - Each output row-group of M=29 oy rows spans 4*28+13 = 125 <= 128 input
    rows, so ONE K=128 matmul window covers ALL its taps for a given kx.
    Four such groups run CONCURRENTLY in the four 32-col PE strips
    (tile_position=(0,32g)), each streaming its own rhs on its own XBUS:
    26 effective N=512 stream slots per 116-oy span (measured ~220-260ns
    per 4-concurrent slot) instead of 104 serial matmuls.
  - Windows start at y0 = 4*oy_start, so the banded weight matrix
    Wb[r, m] = ker[r-4m, kx] is IDENTICAL for all groups/spans: one
    [128, 13*32] fp16 weight tile (106 KB) serves the whole kernel.
  - Image is fp8 e3m4 (measured end-to-end rms rel err 1.44e-2 < 2e-2
    gate); weights stay fp16 (all-fp8 measures 2.02e-2). PE accepts the
    mixed fp16-stationary x fp8-moving matmul exactly.
  - Output staged to fp16 [3, 128, 1024] (one full-partition store per
    span: partial-partition stores trickle at single-descriptor rate),
    upcast and unpacked on host.
  - Spans: oy [0,116) groups at y0=116g; oy [116,232) at y0=464+116g;
    oy [232,256) one M=24 group (K=105) run FIRST as the HAM warm bridge
    while the 8 full image blocks stream in. Span1 drains pair-outer so
    its pair0 store hides under pair1's matmuls.
"""
import numpy as np
import ml_dtypes

import concourse.bacc as bacc
import concourse.mybir as mybir
import concourse.tile as tile
from concourse import bass_utils

KS = 13
PAD = 6
S = 4
B, C, H, W = 8, 4, 1024, 1024
OH = OW = 256
NPH = (W + 2 * PAD) // S  # 259
ROWL = C * S * NPH        # 4144
NROW = H + 2 * PAD        # 1036
M = 29                    # oy rows per column group (4*28+13 = 125 <= 128)
SPAN = 4 * M              # 116
M2 = OH - 2 * SPAN        # 24 (last span, single group)
K2 = 4 * (M2 - 1) + KS    # 105
IDT = mybir.dt.float8e3
NPIDT = ml_dtypes.float8_e3m4

_NC_CACHE = {}


def _host_pack_images(im: np.ndarray) -> np.ndarray:
    """im [8,4,1024,1024] f32 -> [8, 9, 128, ROWL] fp8 row-window tiles.

    tile 0: span2 window rows 928..1032 (105 rows); tiles 1-4: span0 groups
    y0 = 116g; tiles 5-8: span1 groups y0 = 464+116g.
    """
    im_pad = np.pad(im, ((0, 0), (0, 0), (PAD, PAD), (PAD, PAD)), mode="edge")
    planes = im_pad.reshape(B, C, NROW, NPH, S).transpose(0, 1, 2, 4, 3)
    rows = (
        np.ascontiguousarray(planes.transpose(0, 2, 1, 3, 4))
        .reshape(B, NROW, ROWL)
        .astype(NPIDT)
    )
    img = np.zeros((B, 9, 128, ROWL), NPIDT)
    img[:, 0, :K2] = rows[:, 4 * 2 * SPAN : 4 * 2 * SPAN + K2]
    for g in range(4):
        img[:, 1 + g] = rows[:, 4 * M * g : 4 * M * g + 128]
        img[:, 5 + g] = rows[:, 4 * (SPAN + M * g) : 4 * (SPAN + M * g) + 128]
    return img


def _host_pack_weights(kernel: np.ndarray) -> np.ndarray:
    """kernel [8,1,13,13] f32 -> [8, 128, 13*32] fp16 banded matrix.

    w[b, r, kx*32 + m] = kernel[b, 0, r - 4m, kx] (zero outside band).
    Identical band for every group/span since windows start at y0=4*oy0.
    """
    ker = np.asarray(kernel, np.float32)[:, 0]  # [8,13,13]
    r = np.arange(128)[:, None]
    m = np.arange(32)[None, :]
    ky = r - 4 * m
    valid = (ky >= 0) & (ky < KS)
    kyc = np.clip(ky, 0, KS - 1)
    wk = ker[:, kyc].transpose(0, 3, 1, 2)  # [8, 13(kx), 128(r), 32(m)]
    wfull = np.where(valid[None, None], wk, 0.0)
    return (
        np.ascontiguousarray(wfull.transpose(0, 2, 1, 3))
        .reshape(B, 128, KS * 32)
        .astype(np.float16)
    )


def _build_nc():
    nc = bacc.Bacc("TRN2", target_bir_lowering=False, debug=False, num_devices=B)
    img_d = nc.dram_tensor("img", [9, 128, ROWL], IDT, kind="ExternalInput")
    w_d = nc.dram_tensor("w", [128, KS * 32], mybir.dt.float16, kind="ExternalInput")
    out_d = nc.dram_tensor("out", [3, 128, 1024], mybir.dt.float16,
                           kind="ExternalOutput")

    with tile.TileContext(nc) as tc:
        with (
            tc.tile_pool(name="wp", bufs=1) as wp,
            tc.tile_pool(name="ip", bufs=1) as ip,
            tc.tile_pool(name="op", bufs=4) as op,
            tc.tile_pool(name="ps", bufs=4, space="PSUM") as ps,
            tc.tile_pool(name="ps1", bufs=1, space="PSUM") as ps1,
        ):
            wt = wp.tile([128, KS * 32], mybir.dt.float16, tag="w")
            nc.sync.dma_start(wt[:], w_d.ap())

            # image tiles; each split across BOTH HWDGE rings so blocks
            # arrive in strict consumption order at aggregate bandwidth
            half = ROWL // 2
            imgs = []
            for t in range(9):
                tl = ip.tile([128, ROWL], IDT, tag=f"img{t}")
                rows = K2 if t == 0 else 128
                nc.sync.dma_start(tl[0:rows, 0:half], img_d.ap()[t][0:rows, 0:half])
                nc.scalar.dma_start(tl[0:rows, half:], img_d.ap()[t][0:rows, half:])
                imgs.append(tl)

            # PE warm-up against the HAM clock gate while DMAs land
            warm = wp.tile([128, 512], mybir.dt.float16, tag="warm")
            nc.vector.memset(warm[:].bitcast(mybir.dt.uint16), 0)
            pwarm = ps1.tile([128, 512], mybir.dt.float32, tag="pwarm")
            for wi in range(5):
                nc.tensor.matmul(
                    pwarm[:], warm[:, 0:128], warm[:],
                    start=(wi == 0), stop=(wi == 4), skip_group_check=True,
                )

            # ---- span2 first: oy 232..255, one group, K=105 ----
            acc2 = [
                ps1.tile([32, 512], mybir.dt.float32, name=f"acc2_{p}",
                         tag=f"acc2_{p}")
                for p in range(2)
            ]
            rv2 = imgs[0][:].rearrange("p (c x) -> p c x", c=C)
            for kx in range(KS):
                u, s = kx // S, kx % S
                off = s * NPH + u
                for pair in range(2):
                    rhs = rv2[0:K2, 2 * pair : 2 * pair + 2, off : off + 256]
                    nc.tensor.matmul(
                        acc2[pair][0:M2, :],
                        wt[0:K2, kx * 32 : kx * 32 + M2],
                        rhs,
                        start=(kx == 0), stop=(kx == KS - 1),
                        skip_group_check=True,
                    )
            st2 = op.tile([32, 1024], mybir.dt.float16, tag="stage2")
            for pair in range(2):
                nc.vector.tensor_copy(
                    st2[0:M2, 512 * pair : 512 * pair + 512], acc2[pair][0:M2, :]
                )
            nc.scalar.dma_start(out_d.ap()[2][0:M2, :], st2[0:M2, :])

            # ---- spans 0,1: 4 column-tiled groups each ----
            for sp in range(2):
                psums = [
                    ps.tile(
                        [128, 512], mybir.dt.float32,
                        name=f"acc{sp}_{p}", tag="acc",
                    )
                    for p in range(2)
                ]
                rvs = [
                    imgs[1 + 4 * sp + g][:].rearrange("p (c x) -> p c x", c=C)
                    for g in range(4)
                ]
                # span1 pair-outer: pair0's drain overlaps pair1's 13 slots
                if sp == 1:
                    order = [(kx, pair) for pair in range(2) for kx in range(KS)]
                else:
                    order = [(kx, pair) for kx in range(KS) for pair in range(2)]
                n_kx = [0, 0]
                for kx, pair in order:
                    u, s = kx // S, kx % S
                    off = s * NPH + u
                    for g in range(4):
                        rhs = rvs[g][:, 2 * pair : 2 * pair + 2, off : off + 256]
                        nc.tensor.matmul(
                            psums[pair][32 * g : 32 * g + M, :],
                            wt[:, kx * 32 : kx * 32 + M],
                            rhs,
                            start=(n_kx[pair] == 0), stop=(n_kx[pair] == KS - 1),
                            skip_group_check=True,
                            tile_position=(0, 32 * g),
                        )
                    n_kx[pair] += 1
                stage = op.tile([128, 1024], mybir.dt.float16, tag="stage",
                                name=f"stage{sp}")
                for pair in range(2):
                    nc.vector.tensor_copy(
                        stage[:, 512 * pair : 512 * pair + 512], psums[pair][:]
                    )
                    if sp == 1:
                        # store each pair half as soon as its cast lands,
                        # split across both rings
                        nc.sync.dma_start(
                            out_d.ap()[1][0:64, 512 * pair : 512 * pair + 512],
                            stage[0:64, 512 * pair : 512 * pair + 512],
                        )
                        nc.scalar.dma_start(
                            out_d.ap()[1][64:128, 512 * pair : 512 * pair + 512],
                            stage[64:128, 512 * pair : 512 * pair + 512],
                        )
                if sp == 0:
                    nc.sync.dma_start(out_d.ap()[0][0:64, :], stage[0:64, :])
                    nc.scalar.dma_start(out_d.ap()[0][64:128, :], stage[64:128, :])

    nc.compile()
    return nc


def get_nc():
    if "nc" not in _NC_CACHE:
        _NC_CACHE["nc"] = _build_nc()
    return _NC_CACHE["nc"]


def kernel(im, kernel, **run_kwargs):
    im = np.asarray(im, np.float32)
    kernel = np.asarray(kernel, np.float32)
    img = _host_pack_images(im)
    wfull = _host_pack_weights(kernel)
    nc = get_nc()
    in_maps = [{"img": img[b], "w": wfull[b]} for b in range(B)]
    res = bass_utils.run_bass_kernel_spmd(
        nc, in_maps, core_ids=list(range(B)), **run_kwargs
    )
    raw = np.stack([np.asarray(r["out"], np.float32) for r in res.results])
    # raw [B, 3, 128, 1024]: spans 0/1 hold groups at partitions 32g..32g+28
    # (oy = 116*sp + 29g + m); span 2 holds oy 232..255 at partitions 0..23.
    out = np.empty((B, OH, C * OW), np.float32)
    for sp in range(2):
        for g in range(4):
            out[:, SPAN * sp + M * g : SPAN * sp + M * g + M] = (
                raw[:, sp, 32 * g : 32 * g + M]
            )
    out[:, 2 * SPAN :] = raw[:, 2, :M2]
    out = np.ascontiguousarray(
        out.reshape(B, OH, C, OW).transpose(0, 2, 1, 3)
    )
    if run_kwargs:
        return out, res
    return out
